# revision 4
# baseline (speedup 1.0000x reference)
"""Trainium2 Bass kernel: GRU (batch_first, r/z/n gate order) + Bahdanau attention.

Sharding: data-parallel over batch N=8, one batch element per NeuronCore.
Per core:
  - GRU over T_y=300 sequential steps, E=256 hidden. Per step the recurrent
    matmul gh = Whh @ h runs on PE as 12 [128,128]x[128,1] matmuls (LDW-bound),
    gates on DVE/ACT with sigmoid rewritten as 0.5*tanh(x/2)+0.5 so that the
    whole kernel uses one ACT table set (exp_and_others: exp + tanh).
  - Bahdanau attention runs concurrently, lagging the GRU by LAG steps:
    per t: tanh(u.T + w_t) via ACT per-partition bias, v-dot via PE into a
    [1,160] PSUM row, DMA'd into a [128,160] score tile; softmax per 128-t
    chunk with ACT Exp + accum_out.
Host-side prep is layout only: transposes, gate-order reshapes, folding the
bias vectors and the (x2 / x0.5) sigmoid-trick scales into weight copies.
"""

import sys

sys.path.insert(0, "/opt/trn_rl_repo")

from contextlib import ExitStack

import numpy as np

N, TY, TX, E, EH = 8, 300, 160, 256, 128
G3 = 3 * E  # 768
NCORES = 8
LAG = 32

_cache = {}


def _build(debug=False):
    key = bool(debug)
    if key in _cache:
        return _cache[key]

    import concourse.bass as bass  # noqa: F401
    import concourse.tile as tile
    from concourse import bacc, mybir

    FP = mybir.dt.float32
    AF = mybir.ActivationFunctionType
    OP = mybir.AluOpType

    nc = bacc.Bacc("TRN2", target_bir_lowering=False, debug=debug)

    d_xT = nc.dram_tensor("xT", [EH, TY], FP, kind="ExternalInput")
    d_memT = nc.dram_tensor("memT", [E, TX], FP, kind="ExternalInput")
    d_h0 = nc.dram_tensor("h0c", [EH, 2], FP, kind="ExternalInput")
    d_wihT = nc.dram_tensor("wihT", [EH, G3], FP, kind="ExternalInput")
    d_whhT = nc.dram_tensor("whhT", [E, G3], FP, kind="ExternalInput")
    d_bias = nc.dram_tensor("biasc", [EH, 6], FP, kind="ExternalInput")
    d_bhn = nc.dram_tensor("bhnc", [EH, 2], FP, kind="ExternalInput")
    d_wT = nc.dram_tensor("wTT", [E, E], FP, kind="ExternalInput")
    d_uT = nc.dram_tensor("uTT", [E, E], FP, kind="ExternalInput")
    d_v = nc.dram_tensor("vc", [EH, 2], FP, kind="ExternalInput")
    d_id = nc.dram_tensor("ident", [EH, EH], FP, kind="ExternalInput")

    d_attn = nc.dram_tensor("attn", [TY, TX], FP, kind="ExternalOutput")
    d_outs = nc.dram_tensor("outs", [TY, E], FP, kind="ExternalOutput")
    d_hid = nc.dram_tensor("hid", [2, EH], FP, kind="ExternalOutput")

    with tile.TileContext(nc) as tc, ExitStack() as ctx:
        const = ctx.enter_context(tc.tile_pool(name="const", bufs=1))
        wrk = ctx.enter_context(tc.tile_pool(name="wrk", bufs=3))
        ttp = ctx.enter_context(tc.tile_pool(name="ttp", bufs=4))
        scsbp = ctx.enter_context(tc.tile_pool(name="scsb", bufs=2))
        smp = ctx.enter_context(tc.tile_pool(name="smp", bufs=2))
        onatp = ctx.enter_context(tc.tile_pool(name="onat", bufs=2))
        przp = ctx.enter_context(tc.tile_pool(name="prz", bufs=2, space="PSUM"))
        pnp = ctx.enter_context(tc.tile_pool(name="pn", bufs=2, space="PSUM"))
        scpsp = ctx.enter_context(tc.tile_pool(name="scps", bufs=2, space="PSUM"))
        miscp = ctx.enter_context(tc.tile_pool(name="miscps", bufs=2, space="PSUM"))

        # ---- constants into SBUF ----
        whh_sb = [const.tile([EH, G3], FP, tag=f"whh{k}", name=f"whh{k}") for k in range(2)]
        for k in range(2):
            nc.sync.dma_start(whh_sb[k][:], d_whhT[k * EH : (k + 1) * EH, :])
        wih_sb = const.tile([EH, G3], FP, tag="wih")
        nc.sync.dma_start(wih_sb[:], d_wihT[:])
        xT_sb = const.tile([EH, TY], FP, tag="xT")
        nc.sync.dma_start(xT_sb[:], d_xT[:])
        memT_sb = [const.tile([EH, TX], FP, tag=f"memT{k}", name=f"memT{k}") for k in range(2)]
        for k in range(2):
            nc.sync.dma_start(memT_sb[k][:], d_memT[k * EH : (k + 1) * EH, :])
        wT_sb = [const.tile([EH, E], FP, tag=f"wT{k}", name=f"wTt{k}") for k in range(2)]
        uT_sb = [const.tile([EH, E], FP, tag=f"uT{k}", name=f"uTt{k}") for k in range(2)]
        for k in range(2):
            nc.sync.dma_start(wT_sb[k][:], d_wT[k * EH : (k + 1) * EH, :])
            nc.sync.dma_start(uT_sb[k][:], d_uT[k * EH : (k + 1) * EH, :])
        v_sb = const.tile([EH, 2], FP, tag="v")
        nc.sync.dma_start(v_sb[:], d_v[:])
        bias_sb = const.tile([EH, 6], FP, tag="bias")
        nc.sync.dma_start(bias_sb[:], d_bias[:])
        bhn_sb = const.tile([EH, 2], FP, tag="bhn")
        nc.sync.dma_start(bhn_sb[:], d_bhn[:])
        id_sb = const.tile([EH, EH], FP, tag="ident")
        nc.sync.dma_start(id_sb[:], d_id[:])

        Gi = const.tile([EH, 6 * TY], FP, tag="Gi")  # [p, c*300+t]
        outsT = const.tile([EH, 2 * TY], FP, tag="outsT")  # [p, c*300+t]
        w_sb = const.tile([EH, 2 * TY], FP, tag="wsb")  # [p, c*300+t]
        u_sb = [const.tile([EH, TX], FP, tag=f"usb{k}", name=f"usb{k}") for k in range(2)]
        sc_sb = [const.tile([EH, TX], FP, tag=f"scsb{j}", name=f"scsbt{j}") for j in range(3)]

        # ---- Gi = Wih_s @ x.T + bias (per gate-row chunk) ----
        for mc in range(6):
            g_ps = miscp.tile([EH, TY], FP, tag="setup")
            nc.tensor.matmul(
                g_ps[:],
                wih_sb[:, mc * EH : (mc + 1) * EH],
                xT_sb[:],
                start=True,
                stop=True,
            )
            nc.vector.tensor_scalar(
                Gi[:, mc * TY : (mc + 1) * TY],
                g_ps[:],
                bias_sb[:, mc : mc + 1],
                None,
                OP.add,
            )

        # ---- u.T = U @ mem.T ----
        for c in range(2):
            u_ps = miscp.tile([EH, TX], FP, tag="setup")
            for k in range(2):
                nc.tensor.matmul(
                    u_ps[:],
                    uT_sb[k][:, c * EH : (c + 1) * EH],
                    memT_sb[k][:],
                    start=(k == 0),
                    stop=(k == 1),
                )
            nc.vector.tensor_copy(u_sb[c][:], u_ps[:])

        # ---- initial hidden state ----
        h_prev = wrk.tile([EH, 2], FP, tag="h")
        nc.sync.dma_start(h_prev[:], d_h0[:])
        H2_prev = wrk.tile([EH, 2], FP, tag="H2")
        nc.vector.tensor_scalar_mul(H2_prev[:], h_prev[:], 0.5)

        attn_state = {"scp3": None, "g0": 0}

        def emit_attn(tp):
            j, off = tp // EH, tp % EH
            if attn_state["scp3"] is None:
                attn_state["scp3"] = scpsp.tile([1, 3 * TX], FP, tag="scp", name="scp3")
                attn_state["g0"] = tp
            scp3 = attn_state["scp3"]
            slot = tp - attn_state["g0"]
            for c in range(2):
                tt = ttp.tile([EH, TX], FP, tag="tt")
                nc.scalar.activation(
                    tt[:],
                    u_sb[c][:],
                    AF.Tanh,
                    bias=w_sb[:, c * TY + tp : c * TY + tp + 1],
                )
                nc.tensor.matmul(
                    scp3[0:1, slot * TX : (slot + 1) * TX],
                    v_sb[:, c : c + 1],
                    tt[:],
                    start=(c == 0),
                    stop=(c == 1),
                )
            last = EH * j + min(EH, TY - EH * j) - 1
            if slot == 2 or tp == last:
                k = slot + 1
                stage = smp.tile([1, 3 * TX], FP, tag="stage", name="stage")
                nc.vector.tensor_copy(stage[0:1, : k * TX], scp3[0:1, : k * TX])
                o0 = attn_state["g0"] - EH * j
                for i in range(k):
                    nc.sync.dma_start(
                        sc_sb[j][o0 + i : o0 + i + 1, :],
                        stage[0:1, i * TX : (i + 1) * TX],
                    )
                attn_state["scp3"] = None
            if tp == last:
                rows = last - EH * j + 1
                e_sb = smp.tile([EH, TX], FP, tag="esb")
                ssum = smp.tile([EH, 1], FP, tag="ssum")
                nc.scalar.activation(
                    e_sb[:rows, :],
                    sc_sb[j][:rows, :],
                    AF.Exp,
                    accum_out=ssum[:rows, :],
                )
                rec = smp.tile([EH, 1], FP, tag="rec")
                nc.vector.reciprocal(rec[:rows, :], ssum[:rows, :])
                attn_t = smp.tile([EH, TX], FP, tag="attn")
                nc.vector.tensor_scalar(
                    attn_t[:rows, :], e_sb[:rows, :], rec[:rows, :], None, OP.mult
                )
                nc.sync.dma_start(d_attn[EH * j : EH * j + rows, :], attn_t[:rows, :])

        def emit_outs_chunk(j):
            rows = min(EH, TY - EH * j)
            onat = onatp.tile([EH, E], FP, tag="onat")
            for c in range(2):
                t_ps = miscp.tile([EH, EH], FP, tag="setup")
                nc.tensor.transpose(
                    t_ps[:rows, :],
                    outsT[:, c * TY + EH * j : c * TY + EH * j + rows],
                    id_sb[:],
                )
                nc.vector.tensor_copy(
                    onat[:rows, c * EH : (c + 1) * EH], t_ps[:rows, :]
                )
            nc.sync.dma_start(d_outs[EH * j : EH * j + rows, :], onat[:rows, :])

        # ---- GRU loop with interleaved attention ----
        for t in range(TY):
            prz = przp.tile([EH, 4], FP, tag="prz")
            pn = pnp.tile([EH, 2], FP, tag="pn")
            for mc in range(4):
                for k in range(2):
                    nc.tensor.matmul(
                        prz[:, mc : mc + 1],
                        whh_sb[k][:, mc * EH : (mc + 1) * EH],
                        h_prev[:, k : k + 1],
                        start=(k == 0),
                        stop=(k == 1),
                    )
            pre = wrk.tile([EH, 4], FP, tag="pre")
            nc.vector.tensor_add(pre[:], prz[:], Gi[:, t : t + 901 : TY])
            th = wrk.tile([EH, 4], FP, tag="th")
            nc.scalar.activation(th[:], pre[:], AF.Tanh, scale=0.5)
            for mc in range(4, 6):
                for k in range(2):
                    nc.tensor.matmul(
                        pn[:, mc - 4 : mc - 3],
                        whh_sb[k][:, mc * EH : (mc + 1) * EH],
                        h_prev[:, k : k + 1],
                        start=(k == 0),
                        stop=(k == 1),
                    )
            hnb = wrk.tile([EH, 2], FP, tag="hnb")
            nc.vector.tensor_add(hnb[:], pn[:], bhn_sb[:])
            rhn = wrk.tile([EH, 2], FP, tag="rhn")
            nc.vector.scalar_tensor_tensor(
                rhn[:], th[:, 0:2], 1.0, hnb[:], OP.add, OP.mult
            )
            n_in = wrk.tile([EH, 2], FP, tag="nin")
            nc.vector.tensor_add(n_in[:], rhn[:], Gi[:, 1200 + t : 1200 + t + 301 : TY])
            n_t = wrk.tile([EH, 2], FP, tag="nt")
            nc.scalar.activation(n_t[:], n_in[:], AF.Tanh, scale=0.5)
            c_t = wrk.tile([EH, 2], FP, tag="ct")
            nc.vector.tensor_scalar(c_t[:], th[:, 2:4], -0.5, 0.5, OP.mult, OP.add)
            zh = wrk.tile([EH, 2], FP, tag="zh")
            nc.vector.scalar_tensor_tensor(
                zh[:], th[:, 2:4], 1.0, H2_prev[:], OP.add, OP.mult
            )
            g_t = wrk.tile([EH, 2], FP, tag="gt")
            nc.vector.tensor_mul(g_t[:], c_t[:], n_t[:])
            h_new = wrk.tile([EH, 2], FP, tag="h")
            nc.vector.tensor_add(h_new[:], g_t[:], zh[:])
            H2n = wrk.tile([EH, 2], FP, tag="H2")
            nc.vector.tensor_scalar_mul(H2n[:], h_new[:], 0.5)
            nc.vector.tensor_copy(outsT[:, t : t + 301 : TY], h_new[:])
            h_prev, H2_prev = h_new, H2n

            if t % 32 == 31 or t == TY - 1:
                blk = t // 32
                size = t % 32 + 1
                for c in range(2):
                    wps = miscp.tile([EH, 32], FP, tag="setup", name="wps")
                    for k in range(2):
                        nc.tensor.matmul(
                            wps[:, :size],
                            wT_sb[k][:, c * EH : (c + 1) * EH],
                            outsT[:, k * TY + 32 * blk : k * TY + 32 * blk + size],
                            start=(k == 0),
                            stop=(k == 1),
                        )
                    nc.vector.tensor_copy(
                        w_sb[:, c * TY + 32 * blk : c * TY + 32 * blk + size],
                        wps[:, :size],
                    )
            tp = t - LAG
            if tp >= 0:
                emit_attn(tp)
            if t in (EH - 1, 2 * EH - 1, TY - 1):
                emit_outs_chunk(t // EH)

        for tp in range(TY - LAG, TY):
            emit_attn(tp)

        nc.sync.dma_start(d_hid[:].rearrange("c p -> p c"), h_prev[:])

    nc.compile()
    _cache[key] = nc
    return nc


def _prep_in_maps(inputs, memory, h0, Wih, Whh, bih, bhh, W, U, v):
    f32 = np.float32

    def col2(vec):  # [256] -> [128, 2] with [p, c] = vec[c*128+p]
        return np.ascontiguousarray(vec.reshape(2, EH).T, dtype=f32)

    Wih_s = np.array(Wih, dtype=f32).copy()
    Wih_s[2 * E :] *= 2.0
    wihT = np.ascontiguousarray(Wih_s.T)
    whhT = np.ascontiguousarray(np.array(Whh, dtype=f32).T)
    b_rz = (np.array(bih[: 2 * E]) + np.array(bhh[: 2 * E])).astype(f32)
    b2n = (2.0 * np.array(bih[2 * E :])).astype(f32)
    biasc = np.concatenate(
        [b_rz.reshape(4, EH).T, b2n.reshape(2, EH).T], axis=1
    ).astype(f32)
    biasc = np.ascontiguousarray(biasc)
    bhnc = col2(np.array(bhh[2 * E :], dtype=f32))
    wTT = np.ascontiguousarray(np.array(W, dtype=f32).T)
    uTT = np.ascontiguousarray(np.array(U, dtype=f32).T)
    vc = col2(np.array(v, dtype=f32))
    ident = np.eye(EH, dtype=f32)

    shared = dict(
        wihT=wihT, whhT=whhT, biasc=biasc, bhnc=bhnc, wTT=wTT, uTT=uTT, vc=vc,
        ident=ident,
    )
    in_maps = []
    for c in range(N):
        m = dict(shared)
        m["xT"] = np.ascontiguousarray(np.array(inputs[c], dtype=f32).T)
        m["memT"] = np.ascontiguousarray(np.array(memory[c], dtype=f32).T)
        m["h0c"] = col2(np.array(h0[0, c], dtype=f32))
        in_maps.append(m)
    return in_maps


def _assemble(results):
    attn = np.stack([r["attn"] for r in results], axis=0)
    outs = np.stack([r["outs"] for r in results], axis=0)
    hid = np.stack([r["hid"].reshape(E) for r in results], axis=0)[None]
    return attn.astype(np.float32), outs.astype(np.float32), hid.astype(np.float32)


def kernel(inputs, memory, h0, Wih, Whh, bih, bhh, W, U, v, _trace=False):
    from concourse import bass_utils

    nc = _build(debug=False)
    in_maps = _prep_in_maps(inputs, memory, h0, Wih, Whh, bih, bhh, W, U, v)
    res = bass_utils.run_bass_kernel_spmd(
        nc, in_maps, core_ids=list(range(NCORES)), trace=_trace
    )
    out = _assemble(res.results)
    if _trace:
        kernel.last_exec_time_ns = res.exec_time_ns
        kernel.last_results = res
    return out


# revision 8
# speedup vs baseline: 36.1396x; 36.1396x over previous
"""Trainium2 Bass kernel: GRU (batch_first, r/z/n gate order) + Bahdanau attention.

Sharding: data-parallel over batch N=8, one batch element per NeuronCore.
Per core:
  - GRU over T_y=300 sequential steps, E=256 hidden. Per step the recurrent
    matmul gh = Whh @ h runs on PE as 12 [128,128]x[128,1] matmuls (LDW-bound),
    gates on DVE/ACT with sigmoid rewritten as 0.5*tanh(x/2)+0.5 so that the
    whole kernel uses one ACT table set (exp_and_others: exp + tanh).
  - Bahdanau attention runs concurrently, lagging the GRU by LAG steps:
    per t: tanh(u.T + w_t) via ACT per-partition bias, v-dot via PE into a
    [1,160] PSUM row, DMA'd into a [128,160] score tile; softmax per 128-t
    chunk with ACT Exp + accum_out.
Host-side prep is layout only: transposes, gate-order reshapes, folding the
bias vectors and the (x2 / x0.5) sigmoid-trick scales into weight copies.
"""

import sys

sys.path.insert(0, "/opt/trn_rl_repo")

from contextlib import ExitStack

import numpy as np

N, TY, TX, E, EH = 8, 300, 160, 256, 128
G3 = 3 * E  # 768
NCORES = 8
LAG = 32

_cache = {}


def _build(debug=False, loop_n=None):
    key = (bool(debug), loop_n)
    if key in _cache:
        return _cache[key]

    import concourse.bass as bass  # noqa: F401
    import concourse.tile as tile
    from concourse import bacc, mybir

    FP = mybir.dt.float32
    AF = mybir.ActivationFunctionType
    OP = mybir.AluOpType

    nc = bacc.Bacc("TRN2", target_bir_lowering=False, debug=debug)

    d_xT = nc.dram_tensor("xT", [EH, TY], FP, kind="ExternalInput")
    d_memT = nc.dram_tensor("memT", [E, TX], FP, kind="ExternalInput")
    d_h0 = nc.dram_tensor("h0c", [EH, 2], FP, kind="ExternalInput")
    d_wihT = nc.dram_tensor("wihT", [EH, G3], FP, kind="ExternalInput")
    d_whhT = nc.dram_tensor("whhT", [E, G3], FP, kind="ExternalInput")
    d_bias = nc.dram_tensor("biasc", [EH, 6], FP, kind="ExternalInput")
    d_bhn = nc.dram_tensor("bhnc", [EH, 2], FP, kind="ExternalInput")
    d_wT = nc.dram_tensor("wTT", [E, E], FP, kind="ExternalInput")
    d_uT = nc.dram_tensor("uTT", [E, E], FP, kind="ExternalInput")
    d_v = nc.dram_tensor("vc", [EH, 2], FP, kind="ExternalInput")
    d_id = nc.dram_tensor("ident", [EH, EH], FP, kind="ExternalInput")

    d_attn = nc.dram_tensor("attn", [TY, TX], FP, kind="ExternalOutput")
    d_outs = nc.dram_tensor("outs", [TY, E], FP, kind="ExternalOutput")
    d_hid = nc.dram_tensor("hid", [2, EH], FP, kind="ExternalOutput")

    with tile.TileContext(nc) as tc, ExitStack() as ctx:
        if loop_n is not None:
            ctx.enter_context(tc.For_i(0, loop_n, 1))
        const = ctx.enter_context(tc.tile_pool(name="const", bufs=1))
        wrk = ctx.enter_context(tc.tile_pool(name="wrk", bufs=3))
        ttp = ctx.enter_context(tc.tile_pool(name="ttp", bufs=4))
        scsbp = ctx.enter_context(tc.tile_pool(name="scsb", bufs=2))
        smp = ctx.enter_context(tc.tile_pool(name="smp", bufs=2))
        onatp = ctx.enter_context(tc.tile_pool(name="onat", bufs=2))
        przp = ctx.enter_context(tc.tile_pool(name="prz", bufs=2, space="PSUM"))
        pnp = ctx.enter_context(tc.tile_pool(name="pn", bufs=2, space="PSUM"))
        scpsp = ctx.enter_context(tc.tile_pool(name="scps", bufs=2, space="PSUM"))
        miscp = ctx.enter_context(tc.tile_pool(name="miscps", bufs=2, space="PSUM"))

        # ---- constants into SBUF ----
        whh_sb = [const.tile([EH, G3], FP, tag=f"whh{k}", name=f"whh{k}") for k in range(2)]
        for k in range(2):
            nc.sync.dma_start(whh_sb[k][:], d_whhT[k * EH : (k + 1) * EH, :])
        wih_sb = const.tile([EH, G3], FP, tag="wih")
        nc.sync.dma_start(wih_sb[:], d_wihT[:])
        xT_sb = const.tile([EH, TY], FP, tag="xT")
        nc.sync.dma_start(xT_sb[:], d_xT[:])
        memT_sb = [const.tile([EH, TX], FP, tag=f"memT{k}", name=f"memT{k}") for k in range(2)]
        for k in range(2):
            nc.sync.dma_start(memT_sb[k][:], d_memT[k * EH : (k + 1) * EH, :])
        wT_sb = [const.tile([EH, E], FP, tag=f"wT{k}", name=f"wTt{k}") for k in range(2)]
        uT_sb = [const.tile([EH, E], FP, tag=f"uT{k}", name=f"uTt{k}") for k in range(2)]
        for k in range(2):
            nc.sync.dma_start(wT_sb[k][:], d_wT[k * EH : (k + 1) * EH, :])
            nc.sync.dma_start(uT_sb[k][:], d_uT[k * EH : (k + 1) * EH, :])
        v_sb = const.tile([EH, 2], FP, tag="v")
        nc.sync.dma_start(v_sb[:], d_v[:])
        bias_sb = const.tile([EH, 6], FP, tag="bias")
        nc.sync.dma_start(bias_sb[:], d_bias[:])
        bhn_sb = const.tile([EH, 2], FP, tag="bhn")
        nc.sync.dma_start(bhn_sb[:], d_bhn[:])
        id_sb = const.tile([EH, EH], FP, tag="ident")
        nc.sync.dma_start(id_sb[:], d_id[:])

        Gi = const.tile([EH, 6 * TY], FP, tag="Gi")  # [p, c*300+t]
        outsT = const.tile([EH, 2 * TY], FP, tag="outsT")  # [p, c*300+t]
        w_sb = const.tile([EH, 2 * TY], FP, tag="wsb")  # [p, c*300+t]
        u_sb = [const.tile([EH, TX], FP, tag=f"usb{k}", name=f"usb{k}") for k in range(2)]
        sc_sb = [const.tile([EH, TX], FP, tag=f"scsb{j}", name=f"scsbt{j}") for j in range(3)]

        # ---- Gi = Wih_s @ x.T + bias (per gate-row chunk) ----
        for mc in range(6):
            g_ps = miscp.tile([EH, TY], FP, tag="setup")
            nc.tensor.matmul(
                g_ps[:],
                wih_sb[:, mc * EH : (mc + 1) * EH],
                xT_sb[:],
                start=True,
                stop=True,
            )
            nc.vector.tensor_scalar(
                Gi[:, mc * TY : (mc + 1) * TY],
                g_ps[:],
                bias_sb[:, mc : mc + 1],
                None,
                OP.add,
            )

        # ---- u.T = U @ mem.T ----
        for c in range(2):
            u_ps = miscp.tile([EH, TX], FP, tag="setup")
            for k in range(2):
                nc.tensor.matmul(
                    u_ps[:],
                    uT_sb[k][:, c * EH : (c + 1) * EH],
                    memT_sb[k][:],
                    start=(k == 0),
                    stop=(k == 1),
                )
            nc.vector.tensor_copy(u_sb[c][:], u_ps[:])

        # ---- initial hidden state ----
        h_prev = wrk.tile([EH, 2], FP, tag="h")
        nc.sync.dma_start(h_prev[:], d_h0[:])
        H2_prev = wrk.tile([EH, 2], FP, tag="H2")
        nc.vector.tensor_scalar_mul(H2_prev[:], h_prev[:], 0.5)

        attn_state = {"scp3": None, "g0": 0, "stage": None}

        def emit_attn(tp):
            j, off = tp // EH, tp % EH
            if attn_state["stage"] is None:
                attn_state["stage"] = smp.tile(
                    [1, EH * TX], FP, tag="stage", name="stage"
                )
            if attn_state["scp3"] is None:
                attn_state["scp3"] = scpsp.tile([1, 3 * TX], FP, tag="scp", name="scp3")
                attn_state["g0"] = tp
            scp3 = attn_state["scp3"]
            stage = attn_state["stage"]
            slot = tp - attn_state["g0"]
            for c in range(2):
                tt = ttp.tile([EH, TX], FP, tag="tt")
                nc.scalar.activation(
                    tt[:],
                    u_sb[c][:],
                    AF.Tanh,
                    bias=w_sb[:, c * TY + tp : c * TY + tp + 1],
                )
                nc.tensor.matmul(
                    scp3[0:1, slot * TX : (slot + 1) * TX],
                    v_sb[:, c : c + 1],
                    tt[:],
                    start=(c == 0),
                    stop=(c == 1),
                )
            last = EH * j + min(EH, TY - EH * j) - 1
            if slot == 2 or tp == last:
                k = slot + 1
                o0 = attn_state["g0"] - EH * j
                nc.vector.tensor_copy(
                    stage[0:1, o0 * TX : (o0 + k) * TX], scp3[0:1, : k * TX]
                )
                attn_state["scp3"] = None
            if slot == 2 or tp == last:
                o0 = attn_state["g0"] - EH * j
                for i in range(slot + 1):
                    nc.gpsimd.dma_start(
                        sc_sb[j][o0 + i : o0 + i + 1, :],
                        stage[0:1, (o0 + i) * TX : (o0 + i + 1) * TX],
                    )
            if tp == last:
                attn_state["stage"] = None
            if tp == last:
                rows = last - EH * j + 1
                e_sb = smp.tile([EH, TX], FP, tag="esb")
                ssum = smp.tile([EH, 1], FP, tag="ssum")
                nc.scalar.activation(
                    e_sb[:rows, :],
                    sc_sb[j][:rows, :],
                    AF.Exp,
                    accum_out=ssum[:rows, :],
                )
                rec = smp.tile([EH, 1], FP, tag="rec")
                nc.vector.reciprocal(rec[:rows, :], ssum[:rows, :])
                attn_t = smp.tile([EH, TX], FP, tag="attn")
                nc.vector.tensor_scalar(
                    attn_t[:rows, :], e_sb[:rows, :], rec[:rows, :], None, OP.mult
                )
                nc.sync.dma_start(d_attn[EH * j : EH * j + rows, :], attn_t[:rows, :])

        def emit_outs_chunk(j):
            rows = min(EH, TY - EH * j)
            onat = onatp.tile([EH, E], FP, tag="onat")
            for c in range(2):
                t_ps = miscp.tile([EH, EH], FP, tag="setup")
                nc.tensor.transpose(
                    t_ps[:rows, :],
                    outsT[:, c * TY + EH * j : c * TY + EH * j + rows],
                    id_sb[:],
                )
                nc.vector.tensor_copy(
                    onat[:rows, c * EH : (c + 1) * EH], t_ps[:rows, :]
                )
            nc.sync.dma_start(d_outs[EH * j : EH * j + rows, :], onat[:rows, :])

        # ---- GRU loop with interleaved attention ----
        for t in range(TY):
            prz = przp.tile([EH, 4], FP, tag="prz")
            pn = pnp.tile([EH, 2], FP, tag="pn")
            for mc in range(4):
                for k in range(2):
                    nc.tensor.matmul(
                        prz[:, mc : mc + 1],
                        whh_sb[k][:, mc * EH : (mc + 1) * EH],
                        h_prev[:, k : k + 1],
                        start=(k == 0),
                        stop=(k == 1),
                    )
            pre = wrk.tile([EH, 4], FP, tag="pre")
            nc.vector.tensor_add(pre[:], prz[:], Gi[:, t : t + 901 : TY])
            th = wrk.tile([EH, 4], FP, tag="th")
            nc.scalar.activation(th[:], pre[:], AF.Tanh, scale=0.5)
            for mc in range(4, 6):
                for k in range(2):
                    nc.tensor.matmul(
                        pn[:, mc - 4 : mc - 3],
                        whh_sb[k][:, mc * EH : (mc + 1) * EH],
                        h_prev[:, k : k + 1],
                        start=(k == 0),
                        stop=(k == 1),
                    )
            hnb = wrk.tile([EH, 2], FP, tag="hnb")
            nc.vector.tensor_add(hnb[:], pn[:], bhn_sb[:])
            rhn = wrk.tile([EH, 2], FP, tag="rhn")
            nc.vector.scalar_tensor_tensor(
                rhn[:], th[:, 0:2], 1.0, hnb[:], OP.add, OP.mult
            )
            n_in = wrk.tile([EH, 2], FP, tag="nin")
            nc.vector.tensor_add(n_in[:], rhn[:], Gi[:, 1200 + t : 1200 + t + 301 : TY])
            n_t = wrk.tile([EH, 2], FP, tag="nt")
            nc.scalar.activation(n_t[:], n_in[:], AF.Tanh, scale=0.5)
            c_t = wrk.tile([EH, 2], FP, tag="ct")
            nc.vector.tensor_scalar(c_t[:], th[:, 2:4], -0.5, 0.5, OP.mult, OP.add)
            zh = wrk.tile([EH, 2], FP, tag="zh")
            nc.vector.scalar_tensor_tensor(
                zh[:], th[:, 2:4], 1.0, H2_prev[:], OP.add, OP.mult
            )
            g_t = wrk.tile([EH, 2], FP, tag="gt")
            nc.vector.tensor_mul(g_t[:], c_t[:], n_t[:])
            h_new = wrk.tile([EH, 2], FP, tag="h")
            nc.vector.tensor_add(h_new[:], g_t[:], zh[:])
            H2n = wrk.tile([EH, 2], FP, tag="H2")
            nc.vector.tensor_scalar_mul(H2n[:], h_new[:], 0.5)
            nc.vector.tensor_copy(outsT[:, t : t + 301 : TY], h_new[:])
            h_prev, H2_prev = h_new, H2n

            if t % 32 == 31 or t == TY - 1:
                blk = t // 32
                size = t % 32 + 1
                for c in range(2):
                    wps = miscp.tile([EH, 32], FP, tag="setup", name="wps")
                    for k in range(2):
                        nc.tensor.matmul(
                            wps[:, :size],
                            wT_sb[k][:, c * EH : (c + 1) * EH],
                            outsT[:, k * TY + 32 * blk : k * TY + 32 * blk + size],
                            start=(k == 0),
                            stop=(k == 1),
                        )
                    nc.vector.tensor_copy(
                        w_sb[:, c * TY + 32 * blk : c * TY + 32 * blk + size],
                        wps[:, :size],
                    )
            tp = t - LAG
            if tp >= 0:
                emit_attn(tp)
            if t in (EH - 1, 2 * EH - 1, TY - 1):
                emit_outs_chunk(t // EH)

        for tp in range(TY - LAG, TY):
            emit_attn(tp)

        nc.sync.dma_start(d_hid[:].rearrange("c p -> p c"), h_prev[:])

    nc.compile()
    _cache[key] = nc
    return nc


def _prep_in_maps(inputs, memory, h0, Wih, Whh, bih, bhh, W, U, v):
    f32 = np.float32

    def col2(vec):  # [256] -> [128, 2] with [p, c] = vec[c*128+p]
        return np.ascontiguousarray(vec.reshape(2, EH).T, dtype=f32)

    Wih_s = np.array(Wih, dtype=f32).copy()
    Wih_s[2 * E :] *= 2.0
    wihT = np.ascontiguousarray(Wih_s.T)
    whhT = np.ascontiguousarray(np.array(Whh, dtype=f32).T)
    b_rz = (np.array(bih[: 2 * E]) + np.array(bhh[: 2 * E])).astype(f32)
    b2n = (2.0 * np.array(bih[2 * E :])).astype(f32)
    biasc = np.concatenate(
        [b_rz.reshape(4, EH).T, b2n.reshape(2, EH).T], axis=1
    ).astype(f32)
    biasc = np.ascontiguousarray(biasc)
    bhnc = col2(np.array(bhh[2 * E :], dtype=f32))
    wTT = np.ascontiguousarray(np.array(W, dtype=f32).T)
    uTT = np.ascontiguousarray(np.array(U, dtype=f32).T)
    vc = col2(np.array(v, dtype=f32))
    ident = np.eye(EH, dtype=f32)

    shared = dict(
        wihT=wihT, whhT=whhT, biasc=biasc, bhnc=bhnc, wTT=wTT, uTT=uTT, vc=vc,
        ident=ident,
    )
    in_maps = []
    for c in range(N):
        m = dict(shared)
        m["xT"] = np.ascontiguousarray(np.array(inputs[c], dtype=f32).T)
        m["memT"] = np.ascontiguousarray(np.array(memory[c], dtype=f32).T)
        m["h0c"] = col2(np.array(h0[0, c], dtype=f32))
        in_maps.append(m)
    return in_maps


def _assemble(results):
    attn = np.stack([r["attn"] for r in results], axis=0)
    outs = np.stack([r["outs"] for r in results], axis=0)
    hid = np.stack([r["hid"].reshape(E) for r in results], axis=0)[None]
    return attn.astype(np.float32), outs.astype(np.float32), hid.astype(np.float32)


def kernel(inputs, memory, h0, Wih, Whh, bih, bhh, W, U, v, _trace=False):
    from concourse import bass_utils

    nc = _build(debug=False)
    in_maps = _prep_in_maps(inputs, memory, h0, Wih, Whh, bih, bhh, W, U, v)
    res = bass_utils.run_bass_kernel_spmd(
        nc, in_maps, core_ids=list(range(NCORES)), trace=_trace
    )
    out = _assemble(res.results)
    if _trace:
        kernel.last_exec_time_ns = res.exec_time_ns
        kernel.last_results = res
    return out
